# revision 47
# baseline (speedup 1.0000x reference)
"""Multi-head attention kernel for Trainium2, 8 NeuronCores.

Problem: B=4, T=2048, D=1024, H=16 heads, head_dim=64.
Sharding: core c -> batch b = c//2, head group g = c%2 (8 heads each).
Each core computes QKV projections for its 512 features and full
attention for its 8 heads over its batch. No cross-core communication.

Per-core layout (all matmul inputs bf16, fp32 accumulation):
  - x is passed transposed+chunked: xt[p, dc, t] = x[b, t, 128*dc+p]
  - weights passed chunked:  wq[p, dc, f] = Wq[128*dc+p, 512*g+f]
  - Q^T/K^T computed feature-major [feat, t] so attention scores
    S^T[k, q] = sum_d K^T[d, k] Q^T[d, q] come out with k on partitions
  - V computed in natural [t, f] layout, augmented with a ones column:
    PV matmul accumulates [65, 512] where row 64 = softmax denominator
  - softmax needs no max subtraction: |S/8| <= ~7 for N(0,1) inputs
  - output written per head as O^T [64, t]; host transposes/concats
"""

import os
import sys

for _p in ("/opt/trn_rl_repo", "/opt/pypackages"):
    if _p not in sys.path:
        sys.path.insert(0, _p)

import numpy as np
import ml_dtypes

B, T, D, H = 4, 2048, 1024, 16
HD = D // H            # 64 head dim
N_CORES = 8
G = 2                  # head groups (cores per batch)
F = D // G             # 512 features per core
HPC = H // G           # 8 heads per core
P = 128
DC = D // P            # 8 contraction chunks
NPAIR = HPC // 2       # 4 head pairs per core
QC = 512               # query-chunk (columns per score matmul)
NQC = T // QC          # 4 query chunks
NKT = T // P           # 16 key tiles

BF16 = ml_dtypes.bfloat16

_compiled = None  # (nc,) cached across calls in one process


def _build():
    import concourse.bass as bass
    import concourse.tile as tile
    from concourse import bacc, mybir

    fp32 = mybir.dt.float32
    bf16 = mybir.dt.bfloat16
    Exp = mybir.ActivationFunctionType.Exp

    nc = bacc.Bacc("TRN2", target_bir_lowering=False, debug=False,
                   num_devices=N_CORES)

    xt = nc.dram_tensor("xt", [P, DC, T], bf16, kind="ExternalInput").ap()
    # q/k weights are pair-major so the 0.5MB slice pair 0 needs can be
    # DMA'd first (the critical path to the first exp)
    wq = nc.dram_tensor("wq", [P, NPAIR, DC, P], bf16,
                        kind="ExternalInput").ap()
    wk = nc.dram_tensor("wk", [P, NPAIR, DC, P], bf16,
                        kind="ExternalInput").ap()
    wv = nc.dram_tensor("wv", [P, DC, F], bf16, kind="ExternalInput").ap()
    bq = nc.dram_tensor("bq", [P, NPAIR], fp32, kind="ExternalInput").ap()
    bk = nc.dram_tensor("bk", [P, NPAIR], fp32, kind="ExternalInput").ap()
    bv = nc.dram_tensor("bv", [P, F], fp32, kind="ExternalInput").ap()
    o = nc.dram_tensor("o", [HPC, HD, T], fp32, kind="ExternalOutput").ap()

    with tile.TileContext(nc) as tc:
        with (
            tc.tile_pool(name="singles", bufs=1) as singles,
            tc.tile_pool(name="es", bufs=18) as es_pool,
            tc.tile_pool(name="stage", bufs=2) as stage_pool,
            tc.tile_pool(name="norm", bufs=2) as norm_pool,
            tc.tile_pool(name="sps", bufs=2, space="PSUM") as sps_pool,
            tc.tile_pool(name="pv", bufs=1, space="PSUM") as pv_pool,
            tc.tile_pool(name="qkv", bufs=2, space="PSUM") as qkv_pool,
        ):
            # ---- persistent SBUF tensors ----
            xt_sb = singles.tile([P, DC, T], bf16, tag="xt")
            wq_sb = singles.tile([P, NPAIR, DC, P], bf16, tag="wq")
            wk_sb = singles.tile([P, NPAIR, DC, P], bf16, tag="wk")
            wv_sb = singles.tile([P, DC, F], bf16, tag="wv")
            bq_sb = singles.tile([P, NPAIR], fp32, tag="bq")
            bk_sb = singles.tile([P, NPAIR], fp32, tag="bk")
            bv_sb = singles.tile([P, F], fp32, tag="bv")
            # per-pair Q^T/K^T [feat-in-pair, t] and V [t-in-ktile, kt, hp, 65]
            qt_sb = [singles.tile([P, T], bf16, tag=f"qt{j}", name=f"qt{j}")
                     for j in range(NPAIR)]
            kt_sb = [singles.tile([P, T], bf16, tag=f"kt{j}", name=f"kt{j}")
                     for j in range(NPAIR)]
            v_sb = [singles.tile([P, NKT, 2, HD + 1], bf16, tag=f"v{j}",
                                 name=f"v{j}")
                    for j in range(NPAIR)]
            # normalize staging, separate per head-slot (a/b). The [1, 512]
            # Z row would use one DVE lane (3.3us reciprocal), so bounce it
            # through a [128, 4] layout via sb->sb DMA: reciprocal runs on
            # 128 lanes, and the gather-back lands on partition 0 (the only
            # partition gpsimd's partition_broadcast can read on HW).
            zcol = [singles.tile([P, 4], fp32, tag=f"zcol{i}",
                                 name=f"zcol{i}") for i in range(2)]
            rz0 = [singles.tile([1, QC], fp32, tag=f"rz0{i}",
                                name=f"rz0{i}") for i in range(2)]
            rzb = [singles.tile([HD, QC], fp32, tag=f"rzb{i}",
                                name=f"rzb{i}") for i in range(2)]

            # load order matters for startup latency: the first projection
            # chunks need wk + the first xt t-columns, so land those first
            # (one serial queue keeps full HBM bandwidth on the critical
            # chunks; spreading queues was measured slower)
            nc.sync.dma_start(out=wk_sb[:, 0], in_=wk[:, 0])
            nc.sync.dma_start(out=bk_sb[:], in_=bk[:])
            nc.sync.dma_start(out=xt_sb[:, :, 0:512], in_=xt[:, :, 0:512])
            nc.sync.dma_start(out=wq_sb[:, 0], in_=wq[:, 0])
            nc.sync.dma_start(out=bq_sb[:], in_=bq[:])
            for tcn in range(1, T // 512):
                nc.sync.dma_start(out=xt_sb[:, :, 512 * tcn:512 * (tcn + 1)],
                                  in_=xt[:, :, 512 * tcn:512 * (tcn + 1)])
            nc.sync.dma_start(out=wv_sb[:], in_=wv[:])
            nc.sync.dma_start(out=bv_sb[:], in_=bv[:])
            for j in range(1, NPAIR):
                nc.sync.dma_start(out=wk_sb[:, j], in_=wk[:, j])
                nc.sync.dma_start(out=wq_sb[:, j], in_=wq[:, j])
            for j in range(NPAIR):
                nc.vector.memset(v_sb[j][:, :, :, HD:HD + 1], 1.0)
            # warm the ACT exp table (~2.7us load) during the input DMAs
            # instead of on the first real exp
            warm = norm_pool.tile([1, 4], fp32, tag="actwarm", name="actwarm")
            nc.vector.memset(warm[:], 0.0)
            nc.scalar.activation(warm[:], warm[:], Exp, scale=1.0)

            def emit_qk_chunk(j, which, tcn):
                """One [f=128, t=512] t-chunk of Q^T or K^T for pair j."""
                w_sb, b_sb, dst = ((wq_sb, bq_sb, qt_sb[j]) if which == "q"
                                   else (wk_sb, bk_sb, kt_sb[j]))
                ps = qkv_pool.tile([P, 512], fp32, tag="qkv", name="qkps")
                for dc in range(DC):
                    nc.tensor.matmul(
                        ps[:],
                        w_sb[:, j, dc, :],
                        xt_sb[:, dc, 512 * tcn:512 * (tcn + 1)],
                        start=(dc == 0), stop=(dc == DC - 1),
                    )
                nc.vector.tensor_scalar_add(
                    out=dst[:, 512 * tcn:512 * (tcn + 1)],
                    in0=ps[:],
                    scalar1=b_sb[:, j:j + 1],
                )

            def emit_qk_proj(j):
                for which in ("q", "k"):
                    for tcn in range(T // 512):
                        emit_qk_chunk(j, which, tcn)

            def emit_v_proj(tt_lo, tt_hi):
                """V rows, all pairs at once: psum [t=128, f=512] per t-tile."""
                for tt in range(tt_lo, tt_hi):
                    ps = qkv_pool.tile([P, F], fp32, tag="qkv")
                    for dc in range(DC):
                        nc.tensor.matmul(
                            ps[:],
                            xt_sb[:, dc, P * tt:P * (tt + 1)],
                            wv_sb[:, dc, :],
                            start=(dc == 0), stop=(dc == DC - 1),
                        )
                    for j in range(NPAIR):
                        nc.vector.tensor_add(
                            out=v_sb[j][:, tt, :, 0:HD],
                            in0=ps[:, P * j:P * (j + 1)].rearrange(
                                "p (h d) -> p h d", h=2),
                            in1=bv_sb[:, P * j:P * (j + 1)].rearrange(
                                "p (h d) -> p h d", h=2),
                        )

            NTC = T // 512
            # prologue: all of K^T(0) + first chunk of Q^T(0). V is NOT in
            # the prologue: the first iteration's scores/exp only need Q/K,
            # so the scalar engine (the saturated engine) starts ~30us
            # earlier and the V matmuls overlap with the first exps; only
            # that iteration's PV waits for V.
            emit_qk_chunk(0, "k", 0)
            emit_qk_chunk(0, "q", 0)
            for tcn in range(1, NTC):
                emit_qk_chunk(0, "k", tcn)

            def emit_scores_exp(j, qc, ktn):
                qt, kt = qt_sb[j], kt_sb[j]
                q0 = QC * qc
                # scores S^T[k, q] for BOTH heads of the pair in one
                # 2-bank psum tile: head A on PE rows 0-63, head B
                # on rows 64-127. Sharing one tile makes the two
                # matmuls ready simultaneously, so the scheduler
                # keeps them adjacent and the row-disjoint matmuls
                # run concurrently on the array (~2x).
                s = sps_pool.tile([P, 2, QC], fp32, tag="sps", name="s")
                for hp in (0, 1):
                    nc.tensor.matmul(
                        s[:, hp, :],
                        kt[HD * hp:HD * (hp + 1), P * ktn:P * (ktn + 1)],
                        qt[HD * hp:HD * (hp + 1), q0:q0 + QC],
                        start=True, stop=True,
                    )
                es = es_pool.tile([P, 2, QC], bf16, tag="es", name="es")
                nc.scalar.activation(
                    es[:].rearrange("p a b -> p (a b)"),
                    s[:].rearrange("p a b -> p (a b)"),
                    Exp, scale=0.125)
                return es

            def emit_pv(j, qc, ktn, es, pva, pvb):
                vv = v_sb[j]
                first = ktn == 0
                last = ktn == NKT - 1
                nc.tensor.matmul(pva[:], vv[:, ktn, 0, :], es[:, 0, :],
                                 start=first, stop=last)
                nc.tensor.matmul(pvb[:], vv[:, ktn, 1, :], es[:, 1, :],
                                 start=first, stop=last)

            for j in range(NPAIR):
                for qc in range(NQC):
                    q0 = QC * qc
                    pva = pv_pool.tile([HD + 1, QC], fp32, tag="pva")
                    pvb = pv_pool.tile([HD + 1, QC], fp32, tag="pvb")
                    if j == 0 and qc == 0:
                        # first iteration: pipeline the V projection with
                        # the attention — PV for k-tile kt only needs V
                        # t-tile kt, so V tiles are produced just-in-time
                        # while the scalar engine works through the exps
                        for ktn in range(NKT):
                            es = emit_scores_exp(j, qc, ktn)
                            emit_v_proj(ktn, ktn + 1)
                            emit_pv(j, qc, ktn, es, pva, pvb)
                            if ktn == 2 and NTC > 1:
                                # next iteration's Q chunk, early enough
                                # that its psum slot isn't starved by V
                                emit_qk_chunk(0, "q", 1)
                    else:
                        for ktn in range(NKT):
                            # keep the PE warm through ACT-paced stretches:
                            # the last pair has no projection filler left,
                            # so issue tiny throwaway matmuls (HAM
                            # re-throttles the PE clock after ~3.4us of
                            # contiguous idle)
                            if j == NPAIR - 1 and ktn % 2 == 0:
                                dm = qkv_pool.tile([P, 256], fp32,
                                                   tag="qkv", name="warmmm")
                                nc.tensor.matmul(
                                    dm[:], wq_sb[:, 0, 0, :],
                                    xt_sb[:, 0, 0:256],
                                    start=True, stop=True)
                            es = emit_scores_exp(j, qc, ktn)
                            emit_pv(j, qc, ktn, es, pva, pvb)
                    # normalize: row HD of pv holds Z = sum_k exp(s/8).
                    # Copy psum->sbuf first so the PV banks free up fast
                    # (the recip/broadcast chain is slow but off-critical).
                    pvcs = []
                    for hp, pv_t in ((0, pva), (1, pvb)):
                        pvc = norm_pool.tile([HD + 1, QC], fp32,
                                             tag=f"pvc{hp}", name=f"pvc{hp}")
                        nc.vector.tensor_copy(pvc[:], pv_t[:])
                        pvcs.append(pvc)
                    for hp in (0, 1):
                        nc.sync.dma_start(out=zcol[hp][:],
                                          in_=pvcs[hp][HD:HD + 1, :])
                    for hp in (0, 1):
                        nc.vector.reciprocal(zcol[hp][:], zcol[hp][:])
                        nc.sync.dma_start(out=rz0[hp][:], in_=zcol[hp][:])
                    for hp in (0, 1):
                        nc.gpsimd.partition_broadcast(rzb[hp][:], rz0[hp][:])
                        st = stage_pool.tile([HD, QC], fp32, tag=f"st{hp}",
                                             name=f"st{hp}")
                        nc.vector.tensor_mul(st[:], pvcs[hp][0:HD, :],
                                             rzb[hp][:])
                        nc.sync.dma_start(out=o[2 * j + hp, :, q0:q0 + QC],
                                          in_=st[:])
                    # feed the PE pipeline with projection filler for
                    # upcoming iterations: next pair's K^T chunk-by-chunk,
                    # this pair's remaining Q^T chunks just before use, and
                    # next pair's first Q^T chunk at the boundary
                    for tcn in range(qc * NTC // NQC, (qc + 1) * NTC // NQC):
                        if j + 1 < NPAIR:
                            emit_qk_chunk(j + 1, "k", tcn)
                    nxt = (qc + 1) * NTC // NQC
                    if nxt < NTC:
                        if not (j == 0 and qc == 0 and nxt == 1):
                            emit_qk_chunk(j, "q", nxt)  # emitted early above
                    elif j + 1 < NPAIR:
                        emit_qk_chunk(j + 1, "q", 0)

    nc.compile()
    return nc


def _prep_inputs(x, Wq, bq, Wk, bk, Wv, bv):
    """Host-side shard + layout prep. Returns per-core input dicts."""
    in_maps = []
    xt_cache = {}
    w_cache = {}
    for c in range(N_CORES):
        b, g = c // G, c % G
        if b not in xt_cache:
            xtb = np.ascontiguousarray(x[b].T).astype(BF16)      # [D, T]
            xt_cache[b] = np.ascontiguousarray(
                xtb.reshape(DC, P, T).transpose(1, 0, 2))        # [P, DC, T]
        if g not in w_cache:
            def _w(W):
                Wg = W[:, F * g:F * (g + 1)].astype(BF16)        # [D, F]
                return np.ascontiguousarray(
                    Wg.reshape(DC, P, F).transpose(1, 0, 2))     # [P, DC, F]

            def _w_pm(W):
                # pair-major: [P, NPAIR, DC, 128]
                Wg = W[:, F * g:F * (g + 1)].astype(BF16)        # [D, F]
                return np.ascontiguousarray(
                    Wg.reshape(DC, P, NPAIR, P).transpose(1, 2, 0, 3))
            bqg = bq[F * g:F * (g + 1)].astype(np.float32)
            bkg = bk[F * g:F * (g + 1)].astype(np.float32)
            bvg = bv[F * g:F * (g + 1)].astype(np.float32)
            w_cache[g] = {
                "wq": _w_pm(Wq), "wk": _w_pm(Wk), "wv": _w(Wv),
                # [P, NPAIR]: bias for feature 128*j + p
                "bq": np.ascontiguousarray(bqg.reshape(NPAIR, P).T),
                "bk": np.ascontiguousarray(bkg.reshape(NPAIR, P).T),
                # [P, F]: broadcast along partitions
                "bv": np.ascontiguousarray(
                    np.broadcast_to(bvg[None, :], (P, F))),
            }
        in_maps.append({"xt": xt_cache[b], **w_cache[g]})
    return in_maps


def _run(in_maps, trace_dir=None, trace_cores=None):
    from concourse.bass_utils import run_bass_kernel_spmd

    global _compiled
    if _compiled is None:
        _compiled = _build()
    nc = _compiled

    if trace_dir is not None:
        from trn_agent_boot.trn_boot import _ntff_profile_via_ctypes
        hook = _ntff_profile_via_ctypes("/opt/axon/libaxon_pjrt.so")
        with hook(trace_dir, trace_cores):
            res = run_bass_kernel_spmd(nc, in_maps,
                                       core_ids=list(range(N_CORES)))
    else:
        res = run_bass_kernel_spmd(nc, in_maps, core_ids=list(range(N_CORES)))
    return res


def kernel(x, Wq, bq, Wk, bk, Wv, bv, _trace_dir=None, _trace_cores=None):
    x = np.asarray(x, dtype=np.float32)
    in_maps = _prep_inputs(x, np.asarray(Wq), np.asarray(bq), np.asarray(Wk),
                           np.asarray(bk), np.asarray(Wv), np.asarray(bv))
    res = _run(in_maps, _trace_dir, _trace_cores)
    out = np.empty((B, T, D), np.float32)
    for c in range(N_CORES):
        b, g = c // G, c % G
        oc = np.asarray(res.results[c]["o"])          # [HPC, HD, T]
        out[b, :, F * g:F * (g + 1)] = (
            oc.transpose(2, 0, 1).reshape(T, F))
    return out


# revision 48
# speedup vs baseline: 1.0154x; 1.0154x over previous
"""Multi-head attention kernel for Trainium2, 8 NeuronCores.

Problem: B=4, T=2048, D=1024, H=16 heads, head_dim=64.
Sharding: core c -> batch b = c//2, head group g = c%2 (8 heads each).
Each core computes QKV projections for its 512 features and full
attention for its 8 heads over its batch. No cross-core communication.

Per-core layout (all matmul inputs bf16, fp32 accumulation):
  - x is passed transposed+chunked: xt[p, dc, t] = x[b, t, 128*dc+p]
  - weights passed chunked:  wq[p, dc, f] = Wq[128*dc+p, 512*g+f]
  - Q^T/K^T computed feature-major [feat, t] so attention scores
    S^T[k, q] = sum_d K^T[d, k] Q^T[d, q] come out with k on partitions
  - V computed in natural [t, f] layout, augmented with a ones column:
    PV matmul accumulates [65, 512] where row 64 = softmax denominator
  - softmax needs no max subtraction: |S/8| <= ~7 for N(0,1) inputs
  - output written per head as O^T [64, t]; host transposes/concats
"""

import os
import sys

for _p in ("/opt/trn_rl_repo", "/opt/pypackages"):
    if _p not in sys.path:
        sys.path.insert(0, _p)

import numpy as np
import ml_dtypes

B, T, D, H = 4, 2048, 1024, 16
HD = D // H            # 64 head dim
N_CORES = 8
G = 2                  # head groups (cores per batch)
F = D // G             # 512 features per core
HPC = H // G           # 8 heads per core
P = 128
DC = D // P            # 8 contraction chunks
NPAIR = HPC // 2       # 4 head pairs per core
QC = 512               # query-chunk (columns per score matmul)
NQC = T // QC          # 4 query chunks
NKT = T // P           # 16 key tiles

BF16 = ml_dtypes.bfloat16

_compiled = None  # (nc,) cached across calls in one process


def _build():
    import concourse.bass as bass
    import concourse.tile as tile
    from concourse import bacc, mybir

    fp32 = mybir.dt.float32
    bf16 = mybir.dt.bfloat16
    Exp = mybir.ActivationFunctionType.Exp

    nc = bacc.Bacc("TRN2", target_bir_lowering=False, debug=False,
                   num_devices=N_CORES)

    xt = nc.dram_tensor("xt", [P, DC, T], bf16, kind="ExternalInput").ap()
    # q/k weights are pair-major so the 0.5MB slice pair 0 needs can be
    # DMA'd first (the critical path to the first exp)
    wq = nc.dram_tensor("wq", [P, NPAIR, DC, P], bf16,
                        kind="ExternalInput").ap()
    wk = nc.dram_tensor("wk", [P, NPAIR, DC, P], bf16,
                        kind="ExternalInput").ap()
    wv = nc.dram_tensor("wv", [P, DC, F], bf16, kind="ExternalInput").ap()
    bq = nc.dram_tensor("bq", [P, NPAIR], fp32, kind="ExternalInput").ap()
    bk = nc.dram_tensor("bk", [P, NPAIR], fp32, kind="ExternalInput").ap()
    bv = nc.dram_tensor("bv", [P, F], fp32, kind="ExternalInput").ap()
    o = nc.dram_tensor("o", [HPC, HD, T], fp32, kind="ExternalOutput").ap()

    with tile.TileContext(nc) as tc:
        with (
            tc.tile_pool(name="singles", bufs=1) as singles,
            tc.tile_pool(name="es", bufs=18) as es_pool,
            tc.tile_pool(name="stage", bufs=2) as stage_pool,
            tc.tile_pool(name="norm", bufs=2) as norm_pool,
            tc.tile_pool(name="sps", bufs=2, space="PSUM") as sps_pool,
            tc.tile_pool(name="pv", bufs=1, space="PSUM") as pv_pool,
            tc.tile_pool(name="qkv", bufs=2, space="PSUM") as qkv_pool,
        ):
            # ---- persistent SBUF tensors ----
            xt_sb = singles.tile([P, DC, T], bf16, tag="xt")
            wq_sb = singles.tile([P, NPAIR, DC, P], bf16, tag="wq")
            wk_sb = singles.tile([P, NPAIR, DC, P], bf16, tag="wk")
            wv_sb = singles.tile([P, DC, F], bf16, tag="wv")
            bq_sb = singles.tile([P, NPAIR], fp32, tag="bq")
            bk_sb = singles.tile([P, NPAIR], fp32, tag="bk")
            bv_sb = singles.tile([P, F], fp32, tag="bv")
            # per-pair Q^T/K^T [feat-in-pair, t] and V [t-in-ktile, kt, hp, 65]
            qt_sb = [singles.tile([P, T], bf16, tag=f"qt{j}", name=f"qt{j}")
                     for j in range(NPAIR)]
            kt_sb = [singles.tile([P, T], bf16, tag=f"kt{j}", name=f"kt{j}")
                     for j in range(NPAIR)]
            v_sb = [singles.tile([P, NKT, 2, HD + 1], bf16, tag=f"v{j}",
                                 name=f"v{j}")
                    for j in range(NPAIR)]
            # normalize staging, separate per head-slot (a/b). The [1, 512]
            # Z row would use one DVE lane (3.3us reciprocal), so bounce it
            # through a [128, 4] layout via sb->sb DMA: reciprocal runs on
            # 128 lanes, and the gather-back lands on partition 0 (the only
            # partition gpsimd's partition_broadcast can read on HW).
            zcol = [singles.tile([P, 4], fp32, tag=f"zcol{i}",
                                 name=f"zcol{i}") for i in range(2)]
            rz0 = [singles.tile([1, QC], fp32, tag=f"rz0{i}",
                                name=f"rz0{i}") for i in range(2)]
            rzb = [singles.tile([HD, QC], fp32, tag=f"rzb{i}",
                                name=f"rzb{i}") for i in range(2)]

            # load order matters for startup latency: the first projection
            # chunks need wk + the first xt t-columns, so land those first
            # (one serial queue keeps full HBM bandwidth on the critical
            # chunks; spreading queues was measured slower)
            nc.sync.dma_start(out=wk_sb[:, 0], in_=wk[:, 0])
            nc.sync.dma_start(out=bk_sb[:], in_=bk[:])
            nc.sync.dma_start(out=xt_sb[:, :, 0:512], in_=xt[:, :, 0:512])
            nc.sync.dma_start(out=wq_sb[:, 0], in_=wq[:, 0])
            nc.sync.dma_start(out=bq_sb[:], in_=bq[:])
            for tcn in range(1, T // 512):
                nc.sync.dma_start(out=xt_sb[:, :, 512 * tcn:512 * (tcn + 1)],
                                  in_=xt[:, :, 512 * tcn:512 * (tcn + 1)])
            nc.sync.dma_start(out=wv_sb[:], in_=wv[:])
            nc.sync.dma_start(out=bv_sb[:], in_=bv[:])
            for j in range(1, NPAIR):
                nc.sync.dma_start(out=wk_sb[:, j], in_=wk[:, j])
                nc.sync.dma_start(out=wq_sb[:, j], in_=wq[:, j])
            for j in range(NPAIR):
                nc.vector.memset(v_sb[j][:, :, :, HD:HD + 1], 1.0)

            def emit_qk_chunk(j, which, tcn):
                """One [f=128, t=512] t-chunk of Q^T or K^T for pair j."""
                w_sb, b_sb, dst = ((wq_sb, bq_sb, qt_sb[j]) if which == "q"
                                   else (wk_sb, bk_sb, kt_sb[j]))
                ps = qkv_pool.tile([P, 512], fp32, tag="qkv", name="qkps")
                for dc in range(DC):
                    nc.tensor.matmul(
                        ps[:],
                        w_sb[:, j, dc, :],
                        xt_sb[:, dc, 512 * tcn:512 * (tcn + 1)],
                        start=(dc == 0), stop=(dc == DC - 1),
                    )
                nc.vector.tensor_scalar_add(
                    out=dst[:, 512 * tcn:512 * (tcn + 1)],
                    in0=ps[:],
                    scalar1=b_sb[:, j:j + 1],
                )

            def emit_qk_proj(j):
                for which in ("q", "k"):
                    for tcn in range(T // 512):
                        emit_qk_chunk(j, which, tcn)

            def emit_v_proj(tt_lo, tt_hi):
                """V rows, all pairs at once: psum [t=128, f=512] per t-tile."""
                for tt in range(tt_lo, tt_hi):
                    ps = qkv_pool.tile([P, F], fp32, tag="qkv")
                    for dc in range(DC):
                        nc.tensor.matmul(
                            ps[:],
                            xt_sb[:, dc, P * tt:P * (tt + 1)],
                            wv_sb[:, dc, :],
                            start=(dc == 0), stop=(dc == DC - 1),
                        )
                    for j in range(NPAIR):
                        nc.vector.tensor_add(
                            out=v_sb[j][:, tt, :, 0:HD],
                            in0=ps[:, P * j:P * (j + 1)].rearrange(
                                "p (h d) -> p h d", h=2),
                            in1=bv_sb[:, P * j:P * (j + 1)].rearrange(
                                "p (h d) -> p h d", h=2),
                        )

            NTC = T // 512
            # prologue: all of K^T(0) + first chunk of Q^T(0). V is NOT in
            # the prologue: the first iteration's scores/exp only need Q/K,
            # so the scalar engine (the saturated engine) starts ~30us
            # earlier and the V matmuls overlap with the first exps; only
            # that iteration's PV waits for V.
            emit_qk_chunk(0, "k", 0)
            emit_qk_chunk(0, "q", 0)
            for tcn in range(1, NTC):
                emit_qk_chunk(0, "k", tcn)

            def emit_scores_exp(j, qc, ktn):
                qt, kt = qt_sb[j], kt_sb[j]
                q0 = QC * qc
                # scores S^T[k, q] for BOTH heads of the pair in one
                # 2-bank psum tile: head A on PE rows 0-63, head B
                # on rows 64-127. Sharing one tile makes the two
                # matmuls ready simultaneously, so the scheduler
                # keeps them adjacent and the row-disjoint matmuls
                # run concurrently on the array (~2x).
                s = sps_pool.tile([P, 2, QC], fp32, tag="sps", name="s")
                for hp in (0, 1):
                    nc.tensor.matmul(
                        s[:, hp, :],
                        kt[HD * hp:HD * (hp + 1), P * ktn:P * (ktn + 1)],
                        qt[HD * hp:HD * (hp + 1), q0:q0 + QC],
                        start=True, stop=True,
                    )
                es = es_pool.tile([P, 2, QC], bf16, tag="es", name="es")
                nc.scalar.activation(
                    es[:].rearrange("p a b -> p (a b)"),
                    s[:].rearrange("p a b -> p (a b)"),
                    Exp, scale=0.125)
                return es

            def emit_pv(j, qc, ktn, es, pva, pvb):
                vv = v_sb[j]
                first = ktn == 0
                last = ktn == NKT - 1
                nc.tensor.matmul(pva[:], vv[:, ktn, 0, :], es[:, 0, :],
                                 start=first, stop=last)
                nc.tensor.matmul(pvb[:], vv[:, ktn, 1, :], es[:, 1, :],
                                 start=first, stop=last)

            for j in range(NPAIR):
                for qc in range(NQC):
                    q0 = QC * qc
                    pva = pv_pool.tile([HD + 1, QC], fp32, tag="pva")
                    pvb = pv_pool.tile([HD + 1, QC], fp32, tag="pvb")
                    if j == 0 and qc == 0:
                        # first iteration: pipeline the V projection with
                        # the attention — PV for k-tile kt only needs V
                        # t-tile kt, so V tiles are produced just-in-time
                        # while the scalar engine works through the exps
                        for ktn in range(NKT):
                            es = emit_scores_exp(j, qc, ktn)
                            emit_v_proj(ktn, ktn + 1)
                            emit_pv(j, qc, ktn, es, pva, pvb)
                            if ktn == 2 and NTC > 1:
                                # next iteration's Q chunk, early enough
                                # that its psum slot isn't starved by V
                                emit_qk_chunk(0, "q", 1)
                    else:
                        for ktn in range(NKT):
                            # keep the PE warm through ACT-paced stretches:
                            # the last pair has no projection filler left,
                            # so issue tiny throwaway matmuls (HAM
                            # re-throttles the PE clock after ~3.4us of
                            # contiguous idle)
                            if j == NPAIR - 1 and ktn % 2 == 0:
                                dm = qkv_pool.tile([P, 256], fp32,
                                                   tag="qkv", name="warmmm")
                                nc.tensor.matmul(
                                    dm[:], wq_sb[:, 0, 0, :],
                                    xt_sb[:, 0, 0:256],
                                    start=True, stop=True)
                            es = emit_scores_exp(j, qc, ktn)
                            emit_pv(j, qc, ktn, es, pva, pvb)
                    # normalize: row HD of pv holds Z = sum_k exp(s/8).
                    # Copy psum->sbuf first so the PV banks free up fast
                    # (the recip/broadcast chain is slow but off-critical).
                    pvcs = []
                    for hp, pv_t in ((0, pva), (1, pvb)):
                        pvc = norm_pool.tile([HD + 1, QC], fp32,
                                             tag=f"pvc{hp}", name=f"pvc{hp}")
                        nc.vector.tensor_copy(pvc[:], pv_t[:])
                        pvcs.append(pvc)
                    for hp in (0, 1):
                        nc.sync.dma_start(out=zcol[hp][:],
                                          in_=pvcs[hp][HD:HD + 1, :])
                    for hp in (0, 1):
                        nc.vector.reciprocal(zcol[hp][:], zcol[hp][:])
                        nc.sync.dma_start(out=rz0[hp][:], in_=zcol[hp][:])
                    for hp in (0, 1):
                        nc.gpsimd.partition_broadcast(rzb[hp][:], rz0[hp][:])
                        st = stage_pool.tile([HD, QC], fp32, tag=f"st{hp}",
                                             name=f"st{hp}")
                        nc.vector.tensor_mul(st[:], pvcs[hp][0:HD, :],
                                             rzb[hp][:])
                        nc.sync.dma_start(out=o[2 * j + hp, :, q0:q0 + QC],
                                          in_=st[:])
                    # feed the PE pipeline with projection filler for
                    # upcoming iterations: next pair's K^T chunk-by-chunk,
                    # this pair's remaining Q^T chunks just before use, and
                    # next pair's first Q^T chunk at the boundary
                    for tcn in range(qc * NTC // NQC, (qc + 1) * NTC // NQC):
                        if j + 1 < NPAIR:
                            emit_qk_chunk(j + 1, "k", tcn)
                    nxt = (qc + 1) * NTC // NQC
                    if nxt < NTC:
                        if not (j == 0 and qc == 0 and nxt == 1):
                            emit_qk_chunk(j, "q", nxt)  # emitted early above
                    elif j + 1 < NPAIR:
                        emit_qk_chunk(j + 1, "q", 0)

    nc.compile()
    return nc


def _prep_inputs(x, Wq, bq, Wk, bk, Wv, bv):
    """Host-side shard + layout prep. Returns per-core input dicts."""
    in_maps = []
    xt_cache = {}
    w_cache = {}
    for c in range(N_CORES):
        b, g = c // G, c % G
        if b not in xt_cache:
            xtb = np.ascontiguousarray(x[b].T).astype(BF16)      # [D, T]
            xt_cache[b] = np.ascontiguousarray(
                xtb.reshape(DC, P, T).transpose(1, 0, 2))        # [P, DC, T]
        if g not in w_cache:
            def _w(W):
                Wg = W[:, F * g:F * (g + 1)].astype(BF16)        # [D, F]
                return np.ascontiguousarray(
                    Wg.reshape(DC, P, F).transpose(1, 0, 2))     # [P, DC, F]

            def _w_pm(W):
                # pair-major: [P, NPAIR, DC, 128]
                Wg = W[:, F * g:F * (g + 1)].astype(BF16)        # [D, F]
                return np.ascontiguousarray(
                    Wg.reshape(DC, P, NPAIR, P).transpose(1, 2, 0, 3))
            bqg = bq[F * g:F * (g + 1)].astype(np.float32)
            bkg = bk[F * g:F * (g + 1)].astype(np.float32)
            bvg = bv[F * g:F * (g + 1)].astype(np.float32)
            w_cache[g] = {
                "wq": _w_pm(Wq), "wk": _w_pm(Wk), "wv": _w(Wv),
                # [P, NPAIR]: bias for feature 128*j + p
                "bq": np.ascontiguousarray(bqg.reshape(NPAIR, P).T),
                "bk": np.ascontiguousarray(bkg.reshape(NPAIR, P).T),
                # [P, F]: broadcast along partitions
                "bv": np.ascontiguousarray(
                    np.broadcast_to(bvg[None, :], (P, F))),
            }
        in_maps.append({"xt": xt_cache[b], **w_cache[g]})
    return in_maps


def _run(in_maps, trace_dir=None, trace_cores=None):
    from concourse.bass_utils import run_bass_kernel_spmd

    global _compiled
    if _compiled is None:
        _compiled = _build()
    nc = _compiled

    if trace_dir is not None:
        from trn_agent_boot.trn_boot import _ntff_profile_via_ctypes
        hook = _ntff_profile_via_ctypes("/opt/axon/libaxon_pjrt.so")
        with hook(trace_dir, trace_cores):
            res = run_bass_kernel_spmd(nc, in_maps,
                                       core_ids=list(range(N_CORES)))
    else:
        res = run_bass_kernel_spmd(nc, in_maps, core_ids=list(range(N_CORES)))
    return res


def kernel(x, Wq, bq, Wk, bk, Wv, bv, _trace_dir=None, _trace_cores=None):
    x = np.asarray(x, dtype=np.float32)
    in_maps = _prep_inputs(x, np.asarray(Wq), np.asarray(bq), np.asarray(Wk),
                           np.asarray(bk), np.asarray(Wv), np.asarray(bv))
    res = _run(in_maps, _trace_dir, _trace_cores)
    out = np.empty((B, T, D), np.float32)
    for c in range(N_CORES):
        b, g = c // G, c % G
        oc = np.asarray(res.results[c]["o"])          # [HPC, HD, T]
        out[b, :, F * g:F * (g + 1)] = (
            oc.transpose(2, 0, 1).reshape(T, F))
    return out


# revision 49
# speedup vs baseline: 1.0172x; 1.0018x over previous
"""Multi-head attention kernel for Trainium2, 8 NeuronCores.

Problem: B=4, T=2048, D=1024, H=16 heads, head_dim=64.
Sharding: core c -> batch b = c//2, head group g = c%2 (8 heads each).
Each core computes QKV projections for its 512 features and full
attention for its 8 heads over its batch. No cross-core communication.

Per-core layout (all matmul inputs bf16, fp32 accumulation):
  - x is passed transposed+chunked: xt[p, dc, t] = x[b, t, 128*dc+p]
  - weights passed chunked:  wq[p, dc, f] = Wq[128*dc+p, 512*g+f]
  - Q^T/K^T computed feature-major [feat, t] so attention scores
    S^T[k, q] = sum_d K^T[d, k] Q^T[d, q] come out with k on partitions
  - V computed in natural [t, f] layout, augmented with a ones column:
    PV matmul accumulates [65, 512] where row 64 = softmax denominator
  - softmax needs no max subtraction: |S/8| <= ~7 for N(0,1) inputs
  - output written per head as O^T [64, t]; host transposes/concats
"""

import os
import sys

for _p in ("/opt/trn_rl_repo", "/opt/pypackages"):
    if _p not in sys.path:
        sys.path.insert(0, _p)

import numpy as np
import ml_dtypes

B, T, D, H = 4, 2048, 1024, 16
HD = D // H            # 64 head dim
N_CORES = 8
G = 2                  # head groups (cores per batch)
F = D // G             # 512 features per core
HPC = H // G           # 8 heads per core
P = 128
DC = D // P            # 8 contraction chunks
NPAIR = HPC // 2       # 4 head pairs per core
QC = 512               # query-chunk (columns per score matmul)
NQC = T // QC          # 4 query chunks
NKT = T // P           # 16 key tiles

BF16 = ml_dtypes.bfloat16

_compiled = None  # (nc,) cached across calls in one process


def _build():
    import concourse.bass as bass
    import concourse.tile as tile
    from concourse import bacc, mybir

    fp32 = mybir.dt.float32
    bf16 = mybir.dt.bfloat16
    Exp = mybir.ActivationFunctionType.Exp

    nc = bacc.Bacc("TRN2", target_bir_lowering=False, debug=False,
                   num_devices=N_CORES)

    xt = nc.dram_tensor("xt", [P, DC, T], bf16, kind="ExternalInput").ap()
    # q/k weights are pair-major so the 0.5MB slice pair 0 needs can be
    # DMA'd first (the critical path to the first exp)
    wq = nc.dram_tensor("wq", [P, NPAIR, DC, P], bf16,
                        kind="ExternalInput").ap()
    wk = nc.dram_tensor("wk", [P, NPAIR, DC, P], bf16,
                        kind="ExternalInput").ap()
    wv = nc.dram_tensor("wv", [P, DC, F], bf16, kind="ExternalInput").ap()
    bq = nc.dram_tensor("bq", [P, NPAIR], fp32, kind="ExternalInput").ap()
    bk = nc.dram_tensor("bk", [P, NPAIR], fp32, kind="ExternalInput").ap()
    bv = nc.dram_tensor("bv", [P, F], fp32, kind="ExternalInput").ap()
    o = nc.dram_tensor("o", [HPC, HD, T], fp32, kind="ExternalOutput").ap()

    with tile.TileContext(nc) as tc:
        with (
            tc.tile_pool(name="singles", bufs=1) as singles,
            tc.tile_pool(name="es", bufs=18) as es_pool,
            tc.tile_pool(name="stage", bufs=2) as stage_pool,
            tc.tile_pool(name="norm", bufs=2) as norm_pool,
            tc.tile_pool(name="sps", bufs=2, space="PSUM") as sps_pool,
            tc.tile_pool(name="pv", bufs=1, space="PSUM") as pv_pool,
            tc.tile_pool(name="qkv", bufs=2, space="PSUM") as qkv_pool,
        ):
            # ---- persistent SBUF tensors ----
            xt_sb = singles.tile([P, DC, T], bf16, tag="xt")
            wq_sb = singles.tile([P, NPAIR, DC, P], bf16, tag="wq")
            wk_sb = singles.tile([P, NPAIR, DC, P], bf16, tag="wk")
            wv_sb = singles.tile([P, DC, F], bf16, tag="wv")
            bq_sb = singles.tile([P, NPAIR], fp32, tag="bq")
            bk_sb = singles.tile([P, NPAIR], fp32, tag="bk")
            bv_sb = singles.tile([P, F], fp32, tag="bv")
            # per-pair Q^T/K^T [feat-in-pair, t] and V [t-in-ktile, kt, hp, 65]
            qt_sb = [singles.tile([P, T], bf16, tag=f"qt{j}", name=f"qt{j}")
                     for j in range(NPAIR)]
            kt_sb = [singles.tile([P, T], bf16, tag=f"kt{j}", name=f"kt{j}")
                     for j in range(NPAIR)]
            v_sb = [singles.tile([P, NKT, 2, HD + 1], bf16, tag=f"v{j}",
                                 name=f"v{j}")
                    for j in range(NPAIR)]
            # normalize staging, separate per head-slot (a/b). The [1, 512]
            # Z row would use one DVE lane (3.3us reciprocal), so bounce it
            # through a [128, 4] layout via sb->sb DMA: reciprocal runs on
            # 128 lanes, and the gather-back lands on partition 0 (the only
            # partition gpsimd's partition_broadcast can read on HW).
            zcol = [singles.tile([P, 4], fp32, tag=f"zcol{i}",
                                 name=f"zcol{i}") for i in range(2)]
            rz0 = [singles.tile([1, QC], fp32, tag=f"rz0{i}",
                                name=f"rz0{i}") for i in range(2)]
            rzb = [singles.tile([HD, QC], fp32, tag=f"rzb{i}",
                                name=f"rzb{i}") for i in range(2)]

            # load order matters for startup latency: the first projection
            # chunks need wk + the first xt t-columns, so land those first
            # (one serial queue keeps full HBM bandwidth on the critical
            # chunks; spreading queues was measured slower)
            nc.sync.dma_start(out=wk_sb[:, 0], in_=wk[:, 0])
            nc.sync.dma_start(out=bk_sb[:], in_=bk[:])
            nc.sync.dma_start(out=xt_sb[:, :, 0:512], in_=xt[:, :, 0:512])
            nc.sync.dma_start(out=wq_sb[:, 0], in_=wq[:, 0])
            nc.sync.dma_start(out=bq_sb[:], in_=bq[:])
            for tcn in range(1, T // 512):
                nc.sync.dma_start(out=xt_sb[:, :, 512 * tcn:512 * (tcn + 1)],
                                  in_=xt[:, :, 512 * tcn:512 * (tcn + 1)])
            nc.sync.dma_start(out=wv_sb[:], in_=wv[:])
            nc.sync.dma_start(out=bv_sb[:], in_=bv[:])
            for j in range(1, NPAIR):
                nc.sync.dma_start(out=wk_sb[:, j], in_=wk[:, j])
                nc.sync.dma_start(out=wq_sb[:, j], in_=wq[:, j])
            for j in range(NPAIR):
                nc.vector.memset(v_sb[j][:, :, :, HD:HD + 1], 1.0)

            def emit_qk_chunk(j, which, tcn):
                """One [f=128, t=512] t-chunk of Q^T or K^T for pair j."""
                w_sb, b_sb, dst = ((wq_sb, bq_sb, qt_sb[j]) if which == "q"
                                   else (wk_sb, bk_sb, kt_sb[j]))
                ps = qkv_pool.tile([P, 512], fp32, tag="qkv", name="qkps")
                for dc in range(DC):
                    nc.tensor.matmul(
                        ps[:],
                        w_sb[:, j, dc, :],
                        xt_sb[:, dc, 512 * tcn:512 * (tcn + 1)],
                        start=(dc == 0), stop=(dc == DC - 1),
                    )
                nc.vector.tensor_scalar_add(
                    out=dst[:, 512 * tcn:512 * (tcn + 1)],
                    in0=ps[:],
                    scalar1=b_sb[:, j:j + 1],
                )

            def emit_qk_proj(j):
                for which in ("q", "k"):
                    for tcn in range(T // 512):
                        emit_qk_chunk(j, which, tcn)

            def emit_v_proj(tt_lo, tt_hi):
                """V rows, all pairs at once: psum [t=128, f=512] per t-tile."""
                for tt in range(tt_lo, tt_hi):
                    ps = qkv_pool.tile([P, F], fp32, tag="qkv")
                    for dc in range(DC):
                        nc.tensor.matmul(
                            ps[:],
                            xt_sb[:, dc, P * tt:P * (tt + 1)],
                            wv_sb[:, dc, :],
                            start=(dc == 0), stop=(dc == DC - 1),
                        )
                    for j in range(NPAIR):
                        nc.vector.tensor_add(
                            out=v_sb[j][:, tt, :, 0:HD],
                            in0=ps[:, P * j:P * (j + 1)].rearrange(
                                "p (h d) -> p h d", h=2),
                            in1=bv_sb[:, P * j:P * (j + 1)].rearrange(
                                "p (h d) -> p h d", h=2),
                        )

            NTC = T // 512
            # prologue: all of K^T(0) + first chunk of Q^T(0). V is NOT in
            # the prologue: the first iteration's scores/exp only need Q/K,
            # so the scalar engine (the saturated engine) starts ~30us
            # earlier and the V matmuls overlap with the first exps; only
            # that iteration's PV waits for V.
            emit_qk_chunk(0, "k", 0)
            emit_qk_chunk(0, "q", 0)
            for tcn in range(1, NTC):
                emit_qk_chunk(0, "k", tcn)

            def emit_scores_exp(j, qc, ktn):
                qt, kt = qt_sb[j], kt_sb[j]
                q0 = QC * qc
                # scores S^T[k, q] for BOTH heads of the pair in one
                # 2-bank psum tile: head A on PE rows 0-63, head B
                # on rows 64-127. Sharing one tile makes the two
                # matmuls ready simultaneously, so the scheduler
                # keeps them adjacent and the row-disjoint matmuls
                # run concurrently on the array (~2x).
                s = sps_pool.tile([P, 2, QC], fp32, tag="sps", name="s")
                for hp in (0, 1):
                    nc.tensor.matmul(
                        s[:, hp, :],
                        kt[HD * hp:HD * (hp + 1), P * ktn:P * (ktn + 1)],
                        qt[HD * hp:HD * (hp + 1), q0:q0 + QC],
                        start=True, stop=True,
                    )
                es = es_pool.tile([P, 2, QC], bf16, tag="es", name="es")
                nc.scalar.activation(
                    es[:].rearrange("p a b -> p (a b)"),
                    s[:].rearrange("p a b -> p (a b)"),
                    Exp, scale=0.125)
                return es

            def emit_pv(j, qc, ktn, es, pva, pvb):
                vv = v_sb[j]
                first = ktn == 0
                last = ktn == NKT - 1
                nc.tensor.matmul(pva[:], vv[:, ktn, 0, :], es[:, 0, :],
                                 start=first, stop=last)
                nc.tensor.matmul(pvb[:], vv[:, ktn, 1, :], es[:, 1, :],
                                 start=first, stop=last)

            for j in range(NPAIR):
                for qc in range(NQC):
                    q0 = QC * qc
                    pva = pv_pool.tile([HD + 1, QC], fp32, tag="pva")
                    pvb = pv_pool.tile([HD + 1, QC], fp32, tag="pvb")
                    if j == 0 and qc == 0:
                        # first iteration: pipeline the V projection with
                        # the attention — PV for k-tile kt only needs V
                        # t-tile kt, so V tiles are produced just-in-time
                        # while the scalar engine works through the exps
                        # scores/exp emitted one k-tile ahead of V/PV so
                        # each scores pair outranks the V backlog in the
                        # scheduler's priority order and the scalar engine
                        # stays fed through the V projection
                        es_list = []
                        for ktn in range(NKT):
                            es_list.append(emit_scores_exp(j, qc, ktn))
                            if ktn >= 1:
                                emit_v_proj(ktn - 1, ktn)
                                emit_pv(j, qc, ktn - 1, es_list[ktn - 1],
                                        pva, pvb)
                            if ktn == 2 and NTC > 1:
                                # next iteration's Q chunk, early enough
                                # that its psum slot isn't starved by V
                                emit_qk_chunk(0, "q", 1)
                        emit_v_proj(NKT - 1, NKT)
                        emit_pv(j, qc, NKT - 1, es_list[NKT - 1], pva, pvb)
                    else:
                        for ktn in range(NKT):
                            # keep the PE warm through ACT-paced stretches:
                            # the last pair has no projection filler left,
                            # so issue tiny throwaway matmuls (HAM
                            # re-throttles the PE clock after ~3.4us of
                            # contiguous idle)
                            if j == NPAIR - 1 and ktn % 2 == 0:
                                dm = qkv_pool.tile([P, 256], fp32,
                                                   tag="qkv", name="warmmm")
                                nc.tensor.matmul(
                                    dm[:], wq_sb[:, 0, 0, :],
                                    xt_sb[:, 0, 0:256],
                                    start=True, stop=True)
                            es = emit_scores_exp(j, qc, ktn)
                            emit_pv(j, qc, ktn, es, pva, pvb)
                    # normalize: row HD of pv holds Z = sum_k exp(s/8).
                    # Copy psum->sbuf first so the PV banks free up fast
                    # (the recip/broadcast chain is slow but off-critical).
                    pvcs = []
                    for hp, pv_t in ((0, pva), (1, pvb)):
                        pvc = norm_pool.tile([HD + 1, QC], fp32,
                                             tag=f"pvc{hp}", name=f"pvc{hp}")
                        nc.vector.tensor_copy(pvc[:], pv_t[:])
                        pvcs.append(pvc)
                    for hp in (0, 1):
                        nc.sync.dma_start(out=zcol[hp][:],
                                          in_=pvcs[hp][HD:HD + 1, :])
                    for hp in (0, 1):
                        nc.vector.reciprocal(zcol[hp][:], zcol[hp][:])
                        nc.sync.dma_start(out=rz0[hp][:], in_=zcol[hp][:])
                    for hp in (0, 1):
                        nc.gpsimd.partition_broadcast(rzb[hp][:], rz0[hp][:])
                        st = stage_pool.tile([HD, QC], fp32, tag=f"st{hp}",
                                             name=f"st{hp}")
                        nc.vector.tensor_mul(st[:], pvcs[hp][0:HD, :],
                                             rzb[hp][:])
                        nc.sync.dma_start(out=o[2 * j + hp, :, q0:q0 + QC],
                                          in_=st[:])
                    # feed the PE pipeline with projection filler for
                    # upcoming iterations: next pair's K^T chunk-by-chunk,
                    # this pair's remaining Q^T chunks just before use, and
                    # next pair's first Q^T chunk at the boundary
                    for tcn in range(qc * NTC // NQC, (qc + 1) * NTC // NQC):
                        if j + 1 < NPAIR:
                            emit_qk_chunk(j + 1, "k", tcn)
                    nxt = (qc + 1) * NTC // NQC
                    if nxt < NTC:
                        if not (j == 0 and qc == 0 and nxt == 1):
                            emit_qk_chunk(j, "q", nxt)  # emitted early above
                    elif j + 1 < NPAIR:
                        emit_qk_chunk(j + 1, "q", 0)

    nc.compile()
    return nc


def _prep_inputs(x, Wq, bq, Wk, bk, Wv, bv):
    """Host-side shard + layout prep. Returns per-core input dicts."""
    in_maps = []
    xt_cache = {}
    w_cache = {}
    for c in range(N_CORES):
        b, g = c // G, c % G
        if b not in xt_cache:
            xtb = np.ascontiguousarray(x[b].T).astype(BF16)      # [D, T]
            xt_cache[b] = np.ascontiguousarray(
                xtb.reshape(DC, P, T).transpose(1, 0, 2))        # [P, DC, T]
        if g not in w_cache:
            def _w(W):
                Wg = W[:, F * g:F * (g + 1)].astype(BF16)        # [D, F]
                return np.ascontiguousarray(
                    Wg.reshape(DC, P, F).transpose(1, 0, 2))     # [P, DC, F]

            def _w_pm(W):
                # pair-major: [P, NPAIR, DC, 128]
                Wg = W[:, F * g:F * (g + 1)].astype(BF16)        # [D, F]
                return np.ascontiguousarray(
                    Wg.reshape(DC, P, NPAIR, P).transpose(1, 2, 0, 3))
            bqg = bq[F * g:F * (g + 1)].astype(np.float32)
            bkg = bk[F * g:F * (g + 1)].astype(np.float32)
            bvg = bv[F * g:F * (g + 1)].astype(np.float32)
            w_cache[g] = {
                "wq": _w_pm(Wq), "wk": _w_pm(Wk), "wv": _w(Wv),
                # [P, NPAIR]: bias for feature 128*j + p
                "bq": np.ascontiguousarray(bqg.reshape(NPAIR, P).T),
                "bk": np.ascontiguousarray(bkg.reshape(NPAIR, P).T),
                # [P, F]: broadcast along partitions
                "bv": np.ascontiguousarray(
                    np.broadcast_to(bvg[None, :], (P, F))),
            }
        in_maps.append({"xt": xt_cache[b], **w_cache[g]})
    return in_maps


def _run(in_maps, trace_dir=None, trace_cores=None):
    from concourse.bass_utils import run_bass_kernel_spmd

    global _compiled
    if _compiled is None:
        _compiled = _build()
    nc = _compiled

    if trace_dir is not None:
        from trn_agent_boot.trn_boot import _ntff_profile_via_ctypes
        hook = _ntff_profile_via_ctypes("/opt/axon/libaxon_pjrt.so")
        with hook(trace_dir, trace_cores):
            res = run_bass_kernel_spmd(nc, in_maps,
                                       core_ids=list(range(N_CORES)))
    else:
        res = run_bass_kernel_spmd(nc, in_maps, core_ids=list(range(N_CORES)))
    return res


def kernel(x, Wq, bq, Wk, bk, Wv, bv, _trace_dir=None, _trace_cores=None):
    x = np.asarray(x, dtype=np.float32)
    in_maps = _prep_inputs(x, np.asarray(Wq), np.asarray(bq), np.asarray(Wk),
                           np.asarray(bk), np.asarray(Wv), np.asarray(bv))
    res = _run(in_maps, _trace_dir, _trace_cores)
    out = np.empty((B, T, D), np.float32)
    for c in range(N_CORES):
        b, g = c // G, c % G
        oc = np.asarray(res.results[c]["o"])          # [HPC, HD, T]
        out[b, :, F * g:F * (g + 1)] = (
            oc.transpose(2, 0, 1).reshape(T, F))
    return out
